# revision 1
# baseline (speedup 1.0000x reference)
"""Bass/Trainium2 kernel for BoundaryAwareDownConv.

Computation (see reference): for x[B=8, T=8192, D=512] with a space token at
every position t % 8 == 7, pool each 8-token segment by the mean of its 7
non-space tokens -> pooled[B, W=1024, D], then proj = pooled @ w_proj.T +
b_proj, then LayerNorm(D) * gamma + beta.

Sharding: data-parallel over batch, one batch row per NeuronCore (8 cores).
Params are replicated.

Per-core pipeline (x_core [8192, 512] f32):
  - DMA x in 8 chunks of [128 tok, 8 tiles, 512] (2 MB each).
  - Pooling on TensorE: stationary S [128 tok, 32] (cols 0..15 hold 1/7 over
    the 7 valid tokens of each of the 16 segments in a 128-token tile; cols
    16..31 are zero padding so the full 32-partition output slice is written).
    Four col-tiled matmuls (tile_position (0, 32g)) pack four token-tiles'
    pooled outputs into one PSUM bank [128, 512].
  - PSUM -> SBUF copy, then PE transpose of each [128, 128] block; the 64
    real columns per round are compacted into pooledT [d, w] in SBUF.
  - Projection: psum[w 128, dout 512] = sum_k pooledT[dk, w-chunk].T @
    w_projT[dk, :] plus a K=1 matmul that adds b_proj to every partition.
  - LayerNorm on the psum tile via bn_stats/bn_aggr + Sqrt(var+eps) +
    reciprocal, applied with a fused tensor_scalar; gamma/beta applied only
    when they are not the identity (the reference generates ones/zeros).
"""

import numpy as np

B, T, D = 8, 8192, 512
STRIDE = 8
W = T // STRIDE  # 1024
SEGS_PER_TILE = 16  # 128-token tile -> 16 segments
LN_EPS = 1e-5
N_CORES = 8
N_LOADS = 8          # x DMA chunks per core
TILES_PER_LOAD = 8   # 128-token tiles per DMA chunk
VALID = STRIDE - 1   # 7 non-space tokens per segment
# Matmul datapath dtype: float32r streams 1 output row/cycle (vs 4 for f32's
# two-pass LOW_HIGH mode) at ~1.5e-4 matmul relative error (TF32-like
# rounding of the operands; PSUM accumulation stays exact f32).
USE_F32R = True
# Apply the LN affine on the scalar (ACT) engine instead of DVE.
LN_ON_ACT = True


def _patched_tile_context(tile, mybir, ScopedClock):
    """TileContext whose kernel-tail drain carries no sem waits.

    The walrus build in this container rejects sync-wait commands on Drain
    instructions (setupSyncWait<...NO_STRUCT>: "Too many sync wait commands").
    Stock TileContext parks the global-clock catch-up waits on the SP Drain;
    park them on SP nops (one wait each) instead.
    """

    class PatchedTileContext(tile.TileContext):
        def _drain_and_barrier(self, tick_clock, wait_clock):
            required = ScopedClock({None: tick_clock.global_clock})
            carrier = self.nc.sync.nop(nofuse=True)
            wait_clock.add_sem_waits(carrier.ins, required)
            si = carrier.ins.sync_info
            waits = list(si.on_wait) if si is not None else []
            if len(waits) > 1:
                si.on_wait = waits[:1]
                carrier.ins.sync_info = si
                for w in waits[1:]:
                    extra = self.nc.sync.nop(nofuse=True)
                    extra.ins.sync_info = mybir.SyncInfo(on_wait=[w], on_update=[])
            # The carrier nops run earlier on the same (SP) engine, so the
            # drain transitively waits on everything without carrying waits.
            self.nc.sync.drain()
            self.nc.all_engine_barrier()
            assert self.sems is not None
            popped = self.nc._tile_sem_poison_stack.pop()
            assert popped is self._sem_poison
            self.nc.clear_and_free_semaphores(list(self.sems.allocated().values()))
            self.nc.all_engine_barrier()

    return PatchedTileContext


def _split_multi_waits(nc, mybir):
    """Rewrite the scheduled BIR so no instruction carries more than one sync
    wait (and Drain carries none): the walrus build here rejects them
    (setupSyncWait: "Too many sync wait commands"). Surplus waits move onto
    same-engine InstNoOp carriers placed immediately before the instruction —
    same-engine program order preserves the blocking semantics."""
    n = 0
    for fn in nc.m.functions:
        for bb in fn.blocks:
            changed = False
            new_insts = []
            for inst in bb.instructions:
                si = inst.sync_info
                waits = list(si.on_wait) if si is not None else []
                limit = 0 if inst.opcode == "Drain" else 1
                if len(waits) > limit:
                    changed = True
                    for w in waits[limit:]:
                        n += 1
                        new_insts.append(
                            mybir.InstNoOp(
                                name=f"wsplit_{n}_{inst.name}",
                                engine=inst.engine,
                                sync_info=mybir.SyncInfo(on_wait=[w], on_update=[]),
                                bass_nofuse=True,
                            )
                        )
                    si.on_wait = waits[:limit]
                    inst.sync_info = si
                new_insts.append(inst)
            if changed:
                bb.instructions = new_insts


def _build_bass(apply_gamma_beta: bool, split_waits: bool = True, n_loads: int = N_LOADS):
    import concourse.bass as bass
    import concourse.mybir as mybir
    import concourse.tile as tile
    from concourse.bass import ts, ds
    from concourse.vector_clock import ScopedClock

    PatchedTileContext = _patched_tile_context(tile, mybir, ScopedClock)
    f32 = mybir.dt.float32
    fmm = mybir.dt.float32r if USE_F32R else f32

    nc = bass.Bass("TRN2")
    x = nc.dram_tensor("x", [T, D], fmm, kind="ExternalInput")
    wT = nc.dram_tensor("wT", [D, D], fmm, kind="ExternalInput")  # w_proj.T / 7
    bias = nc.dram_tensor("bias", [1, D], fmm, kind="ExternalInput")
    ones1 = nc.dram_tensor("ones1", [1, 128], fmm, kind="ExternalInput")
    ident = nc.dram_tensor("ident", [128, 128], fmm, kind="ExternalInput")
    if apply_gamma_beta:
        gammaB = nc.dram_tensor("gammaB", [128, D], f32, kind="ExternalInput")
        betaB = nc.dram_tensor("betaB", [128, D], f32, kind="ExternalInput")
    out = nc.dram_tensor("out", [W, D], f32, kind="ExternalOutput")

    with PatchedTileContext(nc) as tc:
        with (
            tc.tile_pool(name="singles", bufs=1) as singles,
            tc.tile_pool(name="pool_sb", bufs=8) as pool_sb,
            tc.tile_pool(name="out_sb", bufs=3) as out_sb,
            tc.tile_pool(name="stat", bufs=8) as stat,
            tc.tile_pool(name="ps_t", bufs=4, space="PSUM") as ps_t,
            tc.tile_pool(name="ps_proj", bufs=2, space="PSUM") as ps_proj,
        ):
            # One-time loads (replicated params, helper matrices).
            id_sb = singles.tile([128, 128], fmm)
            nc.sync.dma_start(out=id_sb[:], in_=ident[:, :])
            wt_sb = singles.tile([128, 4, D], fmm)  # [d_lo, d_hi, dout]
            nc.sync.dma_start(
                out=wt_sb[:], in_=wT[:, :].rearrange("(k p) n -> p k n", p=128)
            )
            bias_sb = singles.tile([1, D], fmm)
            nc.sync.dma_start(out=bias_sb[:], in_=bias[:, :])
            ones_sb = singles.tile([1, 128], fmm)
            nc.sync.dma_start(out=ones_sb[:], in_=ones1[:, :])
            eps_sb = singles.tile([128, 1], f32)
            nc.vector.memset(eps_sb[:], LN_EPS)
            if apply_gamma_beta:
                g_sb = singles.tile([128, D], f32)
                nc.sync.dma_start(out=g_sb[:], in_=gammaB[:, :])
                b_sb = singles.tile([128, D], f32)
                nc.sync.dma_start(out=b_sb[:], in_=betaB[:, :])
            # pooledT[d, w] as [d_lo 128, d_hi 4, w 1024]
            pooledT = singles.tile([128, 4, W], fmm)

            # Pooling happens inside the DMA engines: for each 128-segment
            # chunk R, pm_R[seg, d] accumulates the 7 valid token rows via
            # SWDGE accum_op=add (CCE). Emitted j-major within a group of
            # chunks so each chunk's wave-j DMA has long finished by the
            # time its wave-j+1 DMA issues (the Pool sequencer then never
            # stalls on the WAW dependency).
            pms = {}
            group_size = 4
            for g0 in range(0, n_loads, group_size):
                grp = range(g0, min(g0 + group_size, n_loads))
                for j in range(STRIDE - 1):
                    for R in grp:
                        if j == 0:
                            pms[R] = pool_sb.tile([128, D], fmm, name=f"pm{R}")
                        xv = x[R * 1024 : (R + 1) * 1024, :].rearrange(
                            "(s j) d -> s j d", j=STRIDE
                        )
                        nc.gpsimd.dma_start(
                            out=pms[R][:],
                            in_=xv[:, j, :],
                            accum_op=(
                                mybir.AluOpType.bypass
                                if j == 0
                                else mybir.AluOpType.add
                            ),
                        )

            for R in range(n_loads):  # 128 segments / 128 output rows per R
                pm = pms[R]
                # transpose pooled_m -> pooledT columns 128R..128R+127
                for k in range(4):
                    pt = ps_t.tile([128, 128], fmm)
                    nc.tensor.transpose(pt[:], pm[:, ts(k, 128)], id_sb[:])
                    nc.vector.tensor_copy(out=pooledT[:, k, ts(R, 128)], in_=pt[:])
                # projection + bias for w-chunk R
                pp = ps_proj.tile([128, D], f32)
                for k in range(4):
                    nc.tensor.matmul(
                        pp[:],
                        lhsT=pooledT[:, k, ts(R, 128)],
                        rhs=wt_sb[:, k, :],
                        start=(k == 0),
                        stop=False,
                    )
                nc.tensor.matmul(
                    pp[:], lhsT=ones_sb[:], rhs=bias_sb[:], start=False, stop=True
                )
                # LayerNorm: stats on DVE, apply on ACT
                stats = stat.tile([128, 6], f32)
                nc.vector.bn_stats(out=stats[:], in_=pp[:])
                mv = stat.tile([128, 2], f32)
                nc.vector.bn_aggr(out=mv[:], in_=stats[:])
                rstd = stat.tile([128, 1], f32)
                nc.scalar.activation(
                    out=rstd[:],
                    in_=mv[:, 1:2],
                    func=mybir.ActivationFunctionType.Sqrt,
                    bias=eps_sb[:],
                    scale=1.0,
                )
                nc.vector.reciprocal(out=rstd[:], in_=rstd[:])
                ot = out_sb.tile([128, D], f32)
                nmu = stat.tile([128, 1], f32)  # -mu * rstd
                nc.vector.tensor_scalar(
                    out=nmu[:],
                    in0=mv[:, 0:1],
                    scalar1=rstd[:],
                    scalar2=-1.0,
                    op0=mybir.AluOpType.mult,
                    op1=mybir.AluOpType.mult,
                )
                nc.scalar.activation(
                    out=ot[:],
                    in_=pp[:],
                    func=mybir.ActivationFunctionType.Identity,
                    bias=nmu[:],
                    scale=rstd[:],
                )
                if apply_gamma_beta:
                    nc.vector.tensor_mul(out=ot[:], in0=ot[:], in1=g_sb[:])
                    nc.vector.tensor_add(out=ot[:], in0=ot[:], in1=b_sb[:])
                nc.sync.dma_start(out=out[ts(R, 128), :], in_=ot[:])

    if split_waits:
        _split_multi_waits(nc, mybir)
    return nc


def _pool_matrix() -> np.ndarray:
    S = np.zeros((128, 32), dtype=np.float32)
    for m in range(SEGS_PER_TILE):
        for j in range(VALID):
            S[STRIDE * m + j, m] = 1.0 / VALID
    return S


def kernel(**inputs) -> np.ndarray:
    from concourse.bass_utils import run_bass_kernel_spmd

    x = np.asarray(inputs["x"], dtype=np.float32)
    w = np.asarray(inputs["w_proj"], dtype=np.float32)
    b = np.asarray(inputs["b_proj"], dtype=np.float32)
    gamma = np.asarray(inputs["gamma"], dtype=np.float32)
    beta = np.asarray(inputs["beta"], dtype=np.float32)
    assert x.shape == (B, T, D), x.shape

    apply_gb = not (np.all(gamma == 1.0) and np.all(beta == 0.0))
    nc = _build_bass(apply_gb)

    common = {
        "wT": (np.ascontiguousarray(w.T) / VALID).astype(np.float32),
        "bias": np.ascontiguousarray(b.reshape(1, D)),
        "ones1": np.ones((1, 128), dtype=np.float32),
        "ident": np.eye(128, dtype=np.float32),
    }
    if apply_gb:
        common["gammaB"] = np.ascontiguousarray(
            np.broadcast_to(gamma.reshape(1, D), (128, D))
        )
        common["betaB"] = np.ascontiguousarray(
            np.broadcast_to(beta.reshape(1, D), (128, D))
        )

    in_maps = [
        {"x": np.ascontiguousarray(x[i]), **common} for i in range(N_CORES)
    ]
    res = run_bass_kernel_spmd(nc, in_maps, core_ids=list(range(N_CORES)))
    return np.stack([res.results[i]["out"] for i in range(N_CORES)], axis=0)


if __name__ == "__main__":
    rng = np.random.default_rng(0)
    demo = {
        "x": rng.standard_normal((B, T, D), dtype=np.float32),
        "input_ids": np.zeros((B, T), dtype=np.int64),
        "w_proj": rng.standard_normal((D, D), dtype=np.float32) / np.sqrt(D),
        "b_proj": (rng.standard_normal(D) * 0.01).astype(np.float32),
        "gamma": np.ones(D, dtype=np.float32),
        "beta": np.zeros(D, dtype=np.float32),
    }
    out = kernel(**demo)
    print(out.shape, out.dtype, float(np.abs(out).mean()))



# revision 3
# speedup vs baseline: 1.8007x; 1.8007x over previous
"""Bass/Trainium2 kernel for BoundaryAwareDownConv.

Computation (see reference): for x[B=8, T=8192, D=512] with a space token at
every position t % 8 == 7, pool each 8-token segment by the mean of its 7
non-space tokens -> pooled[B, W=1024, D], then proj = pooled @ w_proj.T +
b_proj, then LayerNorm(D) * gamma + beta.

Sharding: data-parallel over batch, one batch row per NeuronCore (8 cores).
Params are replicated.

v2 pipeline (per core, x row staged as fp16 [8192, 512]):
  - 8 chunk DMAs on the SP HWDGE ring, each [128 seg-partitions, 7 tok, 512]
    fp16: partition p reads its segment's 7 non-space token rows, which are
    CONTIGUOUS in DRAM (3.5 KB/partition descriptors) - the space row is
    simply skipped. 7.34 MB total at near line rate, vs the baseline's 56
    SWDGE accumulate-DMAs (~1 us Q7 descriptor-gen each + ~160 ns per 2 KB
    CCE descriptor ~= 124 GB/s effective).
  - Pooling on DVE as a 4-op fp16 add tree (2 elem/cycle 16-bit mode); the
    final add writes the pooled tile as f32r so the PE transpose path keeps
    the baseline-proven dtype. Scale 1/7 is folded into the staged w.
  - PE: 4 [128,128] transposes per chunk (pooled -> pooledT), psum->SBUF
    copies (fp16 cast) on ACT, then 4 K=128 fp16 matmuls pooledT.T @ wT
    accumulate proj[128 seg, 512] in PSUM f32.
  - Bias add (DVE, broadcast-staged b) then LayerNorm: bn_stats/bn_aggr on
    DVE, Sqrt(var+eps) + apply on ACT, fp16 out tile.
  - Output DMAs ride the ACT HWDGE ring (no head-of-line blocking of the SP
    ring's x loads); out is staged fp16 and upcast to f32 on the host.
"""

import numpy as np

B, T, D = 8, 8192, 512
STRIDE = 8
W = T // STRIDE  # 1024
LN_EPS = 1e-5
N_CORES = 8
N_CHUNKS = 8         # 128 segments (= 1024 tokens) per chunk
VALID = STRIDE - 1   # 7 non-space tokens per segment


def _patched_tile_context(tile, mybir, ScopedClock):
    """TileContext whose kernel-tail drain carries no sem waits.

    The walrus build in this container rejects sync-wait commands on Drain
    instructions (setupSyncWait<...NO_STRUCT>: "Too many sync wait commands").
    Stock TileContext parks the global-clock catch-up waits on the SP Drain;
    park them on SP nops (one wait each) instead.
    """

    class PatchedTileContext(tile.TileContext):
        def _drain_and_barrier(self, tick_clock, wait_clock):
            required = ScopedClock({None: tick_clock.global_clock})
            carrier = self.nc.sync.nop(nofuse=True)
            wait_clock.add_sem_waits(carrier.ins, required)
            si = carrier.ins.sync_info
            waits = list(si.on_wait) if si is not None else []
            if len(waits) > 1:
                si.on_wait = waits[:1]
                carrier.ins.sync_info = si
                for w in waits[1:]:
                    extra = self.nc.sync.nop(nofuse=True)
                    extra.ins.sync_info = mybir.SyncInfo(on_wait=[w], on_update=[])
            # The carrier nops run earlier on the same (SP) engine, so the
            # drain transitively waits on everything without carrying waits.
            self.nc.sync.drain()
            self.nc.all_engine_barrier()
            assert self.sems is not None
            popped = self.nc._tile_sem_poison_stack.pop()
            assert popped is self._sem_poison
            self.nc.clear_and_free_semaphores(list(self.sems.allocated().values()))
            self.nc.all_engine_barrier()

    return PatchedTileContext


def _split_multi_waits(nc, mybir):
    """Rewrite the scheduled BIR so no instruction carries more than one sync
    wait (and Drain carries none): the walrus build here rejects them
    (setupSyncWait: "Too many sync wait commands"). Surplus waits move onto
    same-engine InstNoOp carriers placed immediately before the instruction -
    same-engine program order preserves the blocking semantics."""
    n = 0
    for fn in nc.m.functions:
        for bb in fn.blocks:
            changed = False
            new_insts = []
            for inst in bb.instructions:
                si = inst.sync_info
                waits = list(si.on_wait) if si is not None else []
                limit = 0 if inst.opcode == "Drain" else 1
                if len(waits) > limit:
                    changed = True
                    for w in waits[limit:]:
                        n += 1
                        new_insts.append(
                            mybir.InstNoOp(
                                name=f"wsplit_{n}_{inst.name}",
                                engine=inst.engine,
                                sync_info=mybir.SyncInfo(on_wait=[w], on_update=[]),
                                bass_nofuse=True,
                            )
                        )
                    si.on_wait = waits[:limit]
                    inst.sync_info = si
                new_insts.append(inst)
            if changed:
                bb.instructions = new_insts


def _build_bass(apply_gamma_beta: bool, split_waits: bool = True):
    import concourse.bass as bass
    import concourse.mybir as mybir
    import concourse.tile as tile
    from concourse.bass import ts
    from concourse.vector_clock import ScopedClock

    PatchedTileContext = _patched_tile_context(tile, mybir, ScopedClock)
    f32 = mybir.dt.float32
    f32r = mybir.dt.float32r
    f16 = mybir.dt.float16

    nc = bass.Bass("TRN2")
    x = nc.dram_tensor("x", [T, D], f16, kind="ExternalInput")
    wT = nc.dram_tensor("wT", [D, D], f16, kind="ExternalInput")  # w_proj.T / 7
    bias = nc.dram_tensor("bias", [128, D], f32, kind="ExternalInput")
    ident = nc.dram_tensor("ident", [128, 128], f32r, kind="ExternalInput")
    if apply_gamma_beta:
        gammaB = nc.dram_tensor("gammaB", [128, D], f32, kind="ExternalInput")
        betaB = nc.dram_tensor("betaB", [128, D], f32, kind="ExternalInput")
    out = nc.dram_tensor("out", [W, D], f16, kind="ExternalOutput")

    with PatchedTileContext(nc) as tc:
        with (
            tc.tile_pool(name="singles", bufs=1) as singles,
            tc.tile_pool(name="xr_pool", bufs=4) as xr_pool,
            tc.tile_pool(name="t_pool", bufs=2) as t_pool,
            tc.tile_pool(name="uv_pool", bufs=2) as uv_pool,
            tc.tile_pool(name="pm_pool", bufs=2) as pm_pool,
            tc.tile_pool(name="ptT_pool", bufs=2) as ptT_pool,
            tc.tile_pool(name="out_sb", bufs=3) as out_sb,
            tc.tile_pool(name="stat", bufs=8) as stat,
            tc.tile_pool(name="ps_t", bufs=4, space="PSUM") as ps_t,
            tc.tile_pool(name="ps_proj", bufs=2, space="PSUM") as ps_proj,
        ):
            # One-time loads on the ACT (scalar) HWDGE ring so the SP ring
            # starts streaming x immediately.
            id_sb = singles.tile([128, 128], f32r)
            nc.scalar.dma_start(out=id_sb[:], in_=ident[:, :])
            wt_sb = singles.tile([128, 4, D], f16)  # [d_lo, d_hi, dout]
            nc.scalar.dma_start(
                out=wt_sb[:], in_=wT[:, :].rearrange("(k p) n -> p k n", p=128)
            )
            bias_sb = singles.tile([128, D], f32)
            nc.scalar.dma_start(out=bias_sb[:], in_=bias[:, :])
            eps_sb = singles.tile([128, 1], f32)
            nc.vector.memset(eps_sb[:], LN_EPS)
            if apply_gamma_beta:
                g_sb = singles.tile([128, D], f32)
                nc.scalar.dma_start(out=g_sb[:], in_=gammaB[:, :])
                b_sb = singles.tile([128, D], f32)
                nc.scalar.dma_start(out=b_sb[:], in_=betaB[:, :])

            for R in range(N_CHUNKS):
                # x rows for the 128 segments of this chunk; the 7 non-space
                # rows of each segment are contiguous in DRAM (space row
                # skipped by the AP).
                xr = xr_pool.tile([128, VALID, D], f16, name="xr")
                xv = x[R * 1024 : (R + 1) * 1024, :].rearrange(
                    "(s j) d -> s j d", j=STRIDE
                )
                nc.sync.dma_start(out=xr[:], in_=xv[:, 0:VALID, :])

                # Pooling: fp16 add tree on DVE; final add lands f32r.
                with nc.allow_low_precision(reason="fp16 pooling tree"):
                    t = t_pool.tile([128, 3, D], f16, name="t")
                    nc.vector.tensor_add(
                        t[:], xr[:, 0:5:2, :], xr[:, 1:6:2, :]
                    )
                    uv = uv_pool.tile([128, 2, D], f16, name="uv")
                    nc.vector.tensor_add(uv[:, 0, :], t[:, 0, :], t[:, 1, :])
                    nc.vector.tensor_add(uv[:, 1, :], t[:, 2, :], xr[:, 6, :])
                    pm = pm_pool.tile([128, D], f32r, name="pm")
                    nc.vector.tensor_add(pm[:], uv[:, 0, :], uv[:, 1, :])

                # pooled -> pooledT (PE transpose, psum->SBUF copy on ACT)
                ptT = ptT_pool.tile([128, 4, 128], f16, name="ptT")
                for k in range(4):
                    pt = ps_t.tile([128, 128], f32r, name="pt")
                    nc.tensor.transpose(pt[:], pm[:, ts(k, 128)], id_sb[:])
                    nc.scalar.activation(
                        out=ptT[:, k, :],
                        in_=pt[:],
                        func=mybir.ActivationFunctionType.Identity,
                        scale=1.0,
                    )

                # projection for w-chunk R: psum[seg 128, dout 512]
                pp = ps_proj.tile([128, D], f32, name="pp")
                for k in range(4):
                    nc.tensor.matmul(
                        pp[:],
                        lhsT=ptT[:, k, :],
                        rhs=wt_sb[:, k, :],
                        start=(k == 0),
                        stop=(k == 3),
                    )
                nc.vector.tensor_add(pp[:], pp[:], bias_sb[:])

                # LayerNorm: stats on DVE, apply on ACT
                stats = stat.tile([128, 6], f32, name="stats")
                nc.vector.bn_stats(out=stats[:], in_=pp[:])
                mv = stat.tile([128, 2], f32, name="mv")
                nc.vector.bn_aggr(out=mv[:], in_=stats[:])
                rstd = stat.tile([128, 1], f32, name="rstd")
                nc.scalar.activation(
                    out=rstd[:],
                    in_=mv[:, 1:2],
                    func=mybir.ActivationFunctionType.Sqrt,
                    bias=eps_sb[:],
                    scale=1.0,
                )
                nc.vector.reciprocal(out=rstd[:], in_=rstd[:])
                nmu = stat.tile([128, 1], f32, name="nmu")  # -mu * rstd
                nc.vector.tensor_scalar(
                    out=nmu[:],
                    in0=mv[:, 0:1],
                    scalar1=rstd[:],
                    scalar2=-1.0,
                    op0=mybir.AluOpType.mult,
                    op1=mybir.AluOpType.mult,
                )
                if apply_gamma_beta:
                    ot32 = out_sb.tile([128, D], f32, name="ot32")
                    nc.scalar.activation(
                        out=ot32[:],
                        in_=pp[:],
                        func=mybir.ActivationFunctionType.Identity,
                        bias=nmu[:],
                        scale=rstd[:],
                    )
                    nc.vector.tensor_mul(out=ot32[:], in0=ot32[:], in1=g_sb[:])
                    ot = out_sb.tile([128, D], f16, name="ot")
                    nc.vector.tensor_add(out=ot[:], in0=ot32[:], in1=b_sb[:])
                else:
                    ot = out_sb.tile([128, D], f16, name="ot")
                    nc.scalar.activation(
                        out=ot[:],
                        in_=pp[:],
                        func=mybir.ActivationFunctionType.Identity,
                        bias=nmu[:],
                        scale=rstd[:],
                    )
                nc.scalar.dma_start(out=out[ts(R, 128), :], in_=ot[:])

    if split_waits:
        _split_multi_waits(nc, mybir)
    return nc


def _stage_inputs(inputs) -> tuple[bool, list[dict]]:
    """Host-side staging: fp16 x rows per core + replicated params."""
    x = np.asarray(inputs["x"], dtype=np.float32)
    w = np.asarray(inputs["w_proj"], dtype=np.float32)
    b = np.asarray(inputs["b_proj"], dtype=np.float32)
    gamma = np.asarray(inputs["gamma"], dtype=np.float32)
    beta = np.asarray(inputs["beta"], dtype=np.float32)
    assert x.shape == (B, T, D), x.shape

    apply_gb = not (np.all(gamma == 1.0) and np.all(beta == 0.0))
    common = {
        "wT": np.ascontiguousarray(w.T / VALID).astype(np.float16),
        "bias": np.ascontiguousarray(
            np.broadcast_to(b.reshape(1, D), (128, D)).astype(np.float32)
        ),
        "ident": np.eye(128, dtype=np.float32),
    }
    if apply_gb:
        common["gammaB"] = np.ascontiguousarray(
            np.broadcast_to(gamma.reshape(1, D), (128, D))
        )
        common["betaB"] = np.ascontiguousarray(
            np.broadcast_to(beta.reshape(1, D), (128, D))
        )
    x16 = x.astype(np.float16)
    in_maps = [
        {"x": np.ascontiguousarray(x16[i]), **common} for i in range(N_CORES)
    ]
    return apply_gb, in_maps


def kernel(**inputs) -> np.ndarray:
    from concourse.bass_utils import run_bass_kernel_spmd

    apply_gb, in_maps = _stage_inputs(inputs)
    nc = _build_bass(apply_gb)
    res = run_bass_kernel_spmd(nc, in_maps, core_ids=list(range(N_CORES)))
    return np.stack(
        [res.results[i]["out"].astype(np.float32) for i in range(N_CORES)], axis=0
    )


if __name__ == "__main__":
    rng = np.random.default_rng(0)
    demo = {
        "x": rng.standard_normal((B, T, D), dtype=np.float32),
        "input_ids": np.zeros((B, T), dtype=np.int64),
        "w_proj": rng.standard_normal((D, D), dtype=np.float32) / np.sqrt(D),
        "b_proj": (rng.standard_normal(D) * 0.01).astype(np.float32),
        "gamma": np.ones(D, dtype=np.float32),
        "beta": np.zeros(D, dtype=np.float32),
    }
    out = kernel(**demo)
    print(out.shape, out.dtype, float(np.abs(out).mean()))


# revision 9
# speedup vs baseline: 1.9476x; 1.0815x over previous
"""Bass/Trainium2 kernel for BoundaryAwareDownConv.

Computation (see reference): for x[B=8, T=8192, D=512] with a space token at
every position t % 8 == 7, pool each 8-token segment by the mean of its 7
non-space tokens -> pooled[B, W=1024, D], then proj = pooled @ w_proj.T +
b_proj, then LayerNorm(D) * gamma + beta.

Sharding: data-parallel over batch, one batch row per NeuronCore (8 cores).
Params are replicated.

v3 pipeline (per core, x row staged as fp16 [8192, 512]):
  - 8 chunk DMAs on the SP HWDGE ring, each [128 seg-partitions, 8 tok, 512]
    fp16: partition p reads its segment's 8 token rows, a single CONTIGUOUS
    8 KB descriptor (the space row is loaded but excluded from the add tree;
    full-rate descriptors beat the 12.5%-smaller skip-stride pattern).
  - Pooling as a 4-op fp16 add tree: stage 1 (6 rows -> 3) on the otherwise
    idle GpSimd engine, the remaining 3 adds on DVE; the final add writes
    the pooled tile as f32r for the PE transpose path. Scale 1/7 is folded
    into the staged w.
  - PE: 4 [128,128] transposes per chunk into ONE f32r PSUM bank, a single
    ACT copy to SBUF (fp16 cast), then 4 K=128 fp16 matmuls pooledT.T @ wT
    accumulating proj[128 seg, 512] in PSUM f32, plus a K=1 ones x bias_row
    matmul that adds b_proj to every row.
  - LayerNorm: bn_stats/bn_aggr on DVE, Sqrt(var+eps) on ACT, reciprocal +
    -mu*rstd on DVE, fused apply on ACT, fp16 out tile.
  - Emission is software-pipelined with a 2-chunk skew: chunk R's
    load+pool instructions are emitted alongside chunk R-2's
    transpose/proj/LN instructions, so late-stage ops on DVE/ACT never
    block a later chunk's early-stage ops in engine program order.
  - Output DMAs ride the ACT HWDGE ring (no head-of-line blocking of the SP
    ring's x loads); out is staged fp16 and upcast to f32 on the host.
"""

import numpy as np

B, T, D = 8, 8192, 512
STRIDE = 8
W = T // STRIDE  # 1024
LN_EPS = 1e-5
N_CORES = 8
N_CHUNKS = 8         # 128 segments (= 1024 tokens) per chunk
VALID = STRIDE - 1   # 7 non-space tokens per segment


def _patched_tile_context(tile, mybir, ScopedClock):
    """TileContext whose kernel-tail drain carries no sem waits.

    The walrus build in this container rejects sync-wait commands on Drain
    instructions (setupSyncWait<...NO_STRUCT>: "Too many sync wait commands").
    Stock TileContext parks the global-clock catch-up waits on the SP Drain;
    park them on SP nops (one wait each) instead.
    """

    class PatchedTileContext(tile.TileContext):
        def _drain_and_barrier(self, tick_clock, wait_clock):
            required = ScopedClock({None: tick_clock.global_clock})
            carrier = self.nc.sync.nop(nofuse=True)
            wait_clock.add_sem_waits(carrier.ins, required)
            si = carrier.ins.sync_info
            waits = list(si.on_wait) if si is not None else []
            if len(waits) > 1:
                si.on_wait = waits[:1]
                carrier.ins.sync_info = si
                for w in waits[1:]:
                    extra = self.nc.sync.nop(nofuse=True)
                    extra.ins.sync_info = mybir.SyncInfo(on_wait=[w], on_update=[])
            # The carrier nops run earlier on the same (SP) engine, so the
            # drain transitively waits on everything without carrying waits.
            self.nc.sync.drain()
            self.nc.all_engine_barrier()
            assert self.sems is not None
            popped = self.nc._tile_sem_poison_stack.pop()
            assert popped is self._sem_poison
            self.nc.clear_and_free_semaphores(list(self.sems.allocated().values()))
            self.nc.all_engine_barrier()

    return PatchedTileContext


def _split_multi_waits(nc, mybir):
    """Rewrite the scheduled BIR so no instruction carries more than one sync
    wait (and Drain carries none): the walrus build here rejects them
    (setupSyncWait: "Too many sync wait commands"). Surplus waits move onto
    same-engine InstNoOp carriers placed immediately before the instruction -
    same-engine program order preserves the blocking semantics."""
    n = 0
    for fn in nc.m.functions:
        for bb in fn.blocks:
            changed = False
            new_insts = []
            for inst in bb.instructions:
                si = inst.sync_info
                waits = list(si.on_wait) if si is not None else []
                limit = 0 if inst.opcode == "Drain" else 1
                if len(waits) > limit:
                    changed = True
                    for w in waits[limit:]:
                        n += 1
                        new_insts.append(
                            mybir.InstNoOp(
                                name=f"wsplit_{n}_{inst.name}",
                                engine=inst.engine,
                                sync_info=mybir.SyncInfo(on_wait=[w], on_update=[]),
                                bass_nofuse=True,
                            )
                        )
                    si.on_wait = waits[:limit]
                    inst.sync_info = si
                new_insts.append(inst)
            if changed:
                bb.instructions = new_insts


def _build_bass(apply_gamma_beta: bool, split_waits: bool = True):
    import concourse.bass as bass
    import concourse.mybir as mybir
    import concourse.tile as tile
    from concourse.bass import ts
    from concourse.vector_clock import ScopedClock

    PatchedTileContext = _patched_tile_context(tile, mybir, ScopedClock)
    f32 = mybir.dt.float32
    f32r = mybir.dt.float32r
    f16 = mybir.dt.float16

    nc = bass.Bass("TRN2")
    x = nc.dram_tensor("x", [T, D], f16, kind="ExternalInput")
    wT = nc.dram_tensor("wT", [D, D], f16, kind="ExternalInput")  # w_proj.T / 7
    bias = nc.dram_tensor("bias", [1, D], f16, kind="ExternalInput")
    ones1 = nc.dram_tensor("ones1", [1, 128], f16, kind="ExternalInput")
    ident = nc.dram_tensor("ident", [128, 128], f32r, kind="ExternalInput")
    if apply_gamma_beta:
        gammaB = nc.dram_tensor("gammaB", [128, D], f32, kind="ExternalInput")
        betaB = nc.dram_tensor("betaB", [128, D], f32, kind="ExternalInput")
    out = nc.dram_tensor("out", [W, D], f16, kind="ExternalOutput")

    with PatchedTileContext(nc) as tc:
        with (
            tc.tile_pool(name="singles", bufs=1) as singles,
            tc.tile_pool(name="xr_pool", bufs=4) as xr_pool,
            tc.tile_pool(name="t_pool", bufs=2) as t_pool,
            tc.tile_pool(name="uv_pool", bufs=2) as uv_pool,
            tc.tile_pool(name="pm_pool", bufs=4) as pm_pool,
            tc.tile_pool(name="ptT_pool", bufs=2) as ptT_pool,
            tc.tile_pool(name="out_sb", bufs=3) as out_sb,
            tc.tile_pool(name="stat", bufs=8) as stat,
            tc.tile_pool(name="ps_t", bufs=4, space="PSUM") as ps_t,
            tc.tile_pool(name="ps_proj", bufs=2, space="PSUM") as ps_proj,
        ):
            # One-time loads on the ACT (scalar) HWDGE ring so the SP ring
            # starts streaming x immediately.
            id_sb = singles.tile([128, 128], f32r)
            nc.scalar.dma_start(out=id_sb[:], in_=ident[:, :])
            wt_sb = singles.tile([128, 4, D], f16)  # [d_lo, d_hi, dout]
            nc.scalar.dma_start(
                out=wt_sb[:], in_=wT[:, :].rearrange("(k p) n -> p k n", p=128)
            )
            bias_sb = singles.tile([1, D], f16)
            nc.scalar.dma_start(out=bias_sb[:], in_=bias[:, :])
            ones_sb = singles.tile([1, 128], f16)
            nc.scalar.dma_start(out=ones_sb[:], in_=ones1[:, :])
            eps_sb = singles.tile([128, 1], f32)
            nc.vector.memset(eps_sb[:], LN_EPS)
            if apply_gamma_beta:
                g_sb = singles.tile([128, D], f32)
                nc.scalar.dma_start(out=g_sb[:], in_=gammaB[:, :])
                b_sb = singles.tile([128, D], f32)
                nc.scalar.dma_start(out=b_sb[:], in_=betaB[:, :])

            pms = {}

            def stage_a(R):
                # Contiguous chunk load: partition p holds all 8 rows of
                # segment 128*R + p (one 8 KB descriptor per partition).
                xr = xr_pool.tile([128, STRIDE, D], f16, name="xr")
                xv = x[R * 1024 : (R + 1) * 1024, :].rearrange(
                    "(s j) d -> s j d", j=STRIDE
                )
                nc.sync.dma_start(out=xr[:], in_=xv[:, :, :])

                # Pooling: fp16 add tree; the space row (j=7) is excluded.
                # Stage 1 runs on GpSimd, the rest on DVE; final add lands
                # f32r for the PE transpose.
                with nc.allow_low_precision(reason="fp16 pooling tree"):
                    t = t_pool.tile([128, 3, D], f16, name="t")
                    nc.gpsimd.tensor_add(
                        t[:], xr[:, 0:5:2, :], xr[:, 1:6:2, :]
                    )
                    uv = uv_pool.tile([128, 2, D], f16, name="uv")
                    nc.vector.tensor_add(uv[:, 0, :], t[:, 0, :], t[:, 1, :])
                    nc.vector.tensor_add(uv[:, 1, :], t[:, 2, :], xr[:, 6, :])
                    pm = pm_pool.tile([128, D], f32r, name="pm")
                    nc.vector.tensor_add(pm[:], uv[:, 0, :], uv[:, 1, :])
                pms[R] = pm

            def stage_b(R):
                pm = pms.pop(R)
                # pooled -> pooledT: 4 PE transposes into one PSUM bank,
                # one ACT copy to SBUF (fp16 cast).
                ptp = ps_t.tile([128, 4, 128], f32r, name="ptp")
                for k in range(4):
                    nc.tensor.transpose(
                        ptp[:, k, :], pm[:, ts(k, 128)], id_sb[:]
                    )
                ptT = ptT_pool.tile([128, 4, 128], f16, name="ptT")
                nc.scalar.activation(
                    out=ptT[:],
                    in_=ptp[:],
                    func=mybir.ActivationFunctionType.Identity,
                    scale=1.0,
                )

                # projection for w-chunk R: psum[seg 128, dout 512]
                pp = ps_proj.tile([128, D], f32, name="pp")
                for k in range(4):
                    nc.tensor.matmul(
                        pp[:],
                        lhsT=ptT[:, k, :],
                        rhs=wt_sb[:, k, :],
                        start=(k == 0),
                        stop=False,
                    )
                nc.tensor.matmul(
                    pp[:], lhsT=ones_sb[:], rhs=bias_sb[:], start=False, stop=True
                )

                # LayerNorm: stats on DVE, apply on ACT
                stats = stat.tile([128, 6], f32, name="stats")
                nc.vector.bn_stats(out=stats[:], in_=pp[:])
                mv = stat.tile([128, 2], f32, name="mv")
                nc.vector.bn_aggr(out=mv[:], in_=stats[:])
                rstd = stat.tile([128, 1], f32, name="rstd")
                nc.scalar.activation(
                    out=rstd[:],
                    in_=mv[:, 1:2],
                    func=mybir.ActivationFunctionType.Sqrt,
                    bias=eps_sb[:],
                    scale=1.0,
                )
                nc.vector.reciprocal(out=rstd[:], in_=rstd[:])
                nmu = stat.tile([128, 1], f32, name="nmu")  # -mu * rstd
                nc.vector.tensor_scalar(
                    out=nmu[:],
                    in0=mv[:, 0:1],
                    scalar1=rstd[:],
                    scalar2=-1.0,
                    op0=mybir.AluOpType.mult,
                    op1=mybir.AluOpType.mult,
                )
                if apply_gamma_beta:
                    ot32 = out_sb.tile([128, D], f32, name="ot32")
                    nc.scalar.activation(
                        out=ot32[:],
                        in_=pp[:],
                        func=mybir.ActivationFunctionType.Identity,
                        bias=nmu[:],
                        scale=rstd[:],
                    )
                    nc.vector.tensor_mul(out=ot32[:], in0=ot32[:], in1=g_sb[:])
                    ot = out_sb.tile([128, D], f16, name="ot")
                    nc.vector.tensor_add(out=ot[:], in0=ot32[:], in1=b_sb[:])
                else:
                    ot = out_sb.tile([128, D], f16, name="ot")
                    nc.scalar.activation(
                        out=ot[:],
                        in_=pp[:],
                        func=mybir.ActivationFunctionType.Identity,
                        bias=nmu[:],
                        scale=rstd[:],
                    )
                nc.scalar.dma_start(out=out[ts(R, 128), :], in_=ot[:])

            SKEW = 2
            for i in range(N_CHUNKS + SKEW):
                if i < N_CHUNKS:
                    stage_a(i)
                if i >= SKEW:
                    stage_b(i - SKEW)

    if split_waits:
        _split_multi_waits(nc, mybir)
    return nc


def _stage_inputs(inputs) -> tuple[bool, list[dict]]:
    """Host-side staging: fp16 x rows per core + replicated params."""
    x = np.asarray(inputs["x"], dtype=np.float32)
    w = np.asarray(inputs["w_proj"], dtype=np.float32)
    b = np.asarray(inputs["b_proj"], dtype=np.float32)
    gamma = np.asarray(inputs["gamma"], dtype=np.float32)
    beta = np.asarray(inputs["beta"], dtype=np.float32)
    assert x.shape == (B, T, D), x.shape

    apply_gb = not (np.all(gamma == 1.0) and np.all(beta == 0.0))
    common = {
        "wT": np.ascontiguousarray(w.T / VALID).astype(np.float16),
        "bias": np.ascontiguousarray(b.reshape(1, D)).astype(np.float16),
        "ones1": np.ones((1, 128), dtype=np.float16),
        "ident": np.eye(128, dtype=np.float32),
    }
    if apply_gb:
        common["gammaB"] = np.ascontiguousarray(
            np.broadcast_to(gamma.reshape(1, D), (128, D))
        )
        common["betaB"] = np.ascontiguousarray(
            np.broadcast_to(beta.reshape(1, D), (128, D))
        )
    x16 = x.astype(np.float16)
    in_maps = [
        {"x": np.ascontiguousarray(x16[i]), **common} for i in range(N_CORES)
    ]
    return apply_gb, in_maps


def kernel(**inputs) -> np.ndarray:
    from concourse.bass_utils import run_bass_kernel_spmd

    apply_gb, in_maps = _stage_inputs(inputs)
    nc = _build_bass(apply_gb)
    res = run_bass_kernel_spmd(nc, in_maps, core_ids=list(range(N_CORES)))
    return np.stack(
        [res.results[i]["out"].astype(np.float32) for i in range(N_CORES)], axis=0
    )


if __name__ == "__main__":
    rng = np.random.default_rng(0)
    demo = {
        "x": rng.standard_normal((B, T, D), dtype=np.float32),
        "input_ids": np.zeros((B, T), dtype=np.int64),
        "w_proj": rng.standard_normal((D, D), dtype=np.float32) / np.sqrt(D),
        "b_proj": (rng.standard_normal(D) * 0.01).astype(np.float32),
        "gamma": np.ones(D, dtype=np.float32),
        "beta": np.zeros(D, dtype=np.float32),
    }
    out = kernel(**demo)
    print(out.shape, out.dtype, float(np.abs(out).mean()))
